# revision 42
# baseline (speedup 1.0000x reference)
"""ANI-style per-species MLP (384->160->128->96->1, CELU) over [B=128, A=512]
atoms with species routing, atom-summed to [B]. 8-core SPMD Trainium2 kernel.

Sharding: atom-parallel. Atoms are grouped by species and dealt round-robin to
the 8 cores (padded with zero-AEV dummy atoms whose contribution is
subtracted on the host). Each core streams its [384, S*128] transposed AEV
block in bf16, runs the 4 layers with per-species weights, and emits a
per-molecule partial sum; the host adds the 8 partials.

Kernel structure (per core):
- bf16 matmul datapath (fp32 PSUM), 8-atom tiles = [128, 1024] psum tiles.
- The H0=160 output splits 128+32; the 32-row "spill" psums of a tile's two
  halves are partition-packed into one [64, 512] bank so their CELU merges.
- 3-stage skewed software pipeline: step i runs L0(i), L1(i-1), L2(i-2),
  L3(i-3) on the PE so every celu chain (exp -> clamp -> stt, ~2.5us) hides
  under ~4.4us of matmul work from neighboring tiles. PSUM: p0/p1/p2 tags
  (2 banks each) + spill + L3 accumulator = 8 banks exactly.
- CELU (structure A, sites L0/L2):   e = exp(10z + (10b+ln a))  [ACT]
  e' = min(e-a,0) [DVE 4x bf16]; y = (z max -b)+e' [DVE stt] = celu(t)-b.
  Structure B (site L1, balances ACT/DVE): r = relu(z+c1) [ACT],
  y = r + e' [DVE tensor_tensor] = celu(t); bias folds adjust on the host.
- Species-major tile order so consecutive matmuls share stationary weights;
  small tiles placed first/last to shorten pipeline fill/drain; startup DMAs
  split across the SP and ACT hardware DGE queues.
"""

import os
import sys

import numpy as np

try:
    import concourse  # noqa: F401
except ImportError:
    sys.path.insert(0, "/opt/trn_rl_repo")

N_CORES = 8
B, A, FEAT = 128, 512, 384
N_SPECIES = 4
H0, H1, H2 = 160, 128, 96
ALPHA = 0.1
LNA = float(np.log(ALPHA))

WPS = 833  # weight-pack columns per species (bf16)
CPS = 8    # constant-pack columns per species (fp32)

TRACE = bool(int(os.environ.get("BASSNN_TRACE", "0")))
LAST = {}

_progs = {}


def _maybe_register_ntff_hook():
    try:
        import types

        import antenv
        from antenv import axon_hooks  # noqa: F401
        return
    except ImportError:
        pass
    try:
        import types

        import antenv
        from trn_agent_boot.trn_boot import _ntff_profile_via_ctypes

        mod = types.ModuleType("antenv.axon_hooks")
        holder = [None]
        mod.set_axon_ntff_profile_hook = lambda h: holder.__setitem__(0, h)
        mod.get_axon_ntff_profile_hook = lambda: holder[0]
        sys.modules["antenv.axon_hooks"] = mod
        antenv.axon_hooks = mod
        mod.set_axon_ntff_profile_hook(
            _ntff_profile_via_ctypes("/opt/axon/libaxon_pjrt.so")
        )
    except Exception:
        pass


def _tiles_for_groups(G):
    """Per-species padded group sizes -> list of (species, slot0, n_atoms),
    8-atom tiles with a possibly smaller (even) tail tile."""
    tiles = []
    slot0 = 0
    for s, g in enumerate(G):
        a = 0
        while a < g:
            na = 8 if g - a >= 8 else g - a
            tiles.append((s, slot0 + a, na))
            a += na
        slot0 += g
    return tiles


def _build_program(G, S):
    import concourse.bass as bass
    import concourse.tile as tile
    from concourse import bacc, mybir

    F32 = mybir.dt.float32
    BF16 = mybir.dt.bfloat16
    EXP = mybir.ActivationFunctionType.Exp
    MIN = mybir.AluOpType.min
    MAX = mybir.AluOpType.max
    ADD = mybir.AluOpType.add
    SUB = mybir.AluOpType.subtract

    tiles = _tiles_for_groups(G)
    # order tiles: small tile first (shortens pipeline fill), full tiles,
    # small tiles last (shortens drain). One full tile is split into two
    # 4-atom tiles to provide the small first/last tiles.
    full = [t for t in tiles if t[2] == 8]
    part = [t for t in tiles if t[2] < 8]
    if full:
        s_, a0_, _ = full.pop()
        first, last = (s_, a0_, 4), (s_, a0_ + 4, 4)
        tiles = [first] + full + [last] + part
    ntiles = len(tiles)

    nc = bacc.Bacc("TRN2", target_bir_lowering=False, debug=False,
                   num_devices=N_CORES)
    # x transposed: [feat-part 128, chunk 3, atom-slot S, mol 128] bf16
    xt = nc.dram_tensor("xt", [128, 3, S, 128], BF16, kind="ExternalInput").ap()
    wp = nc.dram_tensor("wp", [128, WPS * N_SPECIES], BF16,
                        kind="ExternalInput").ap()
    cp = nc.dram_tensor("cp", [128, CPS * N_SPECIES], F32,
                        kind="ExternalInput").ap()
    yo = nc.dram_tensor("yo", [1, 128], F32, kind="ExternalOutput").ap()

    from contextlib import ExitStack

    with tile.TileContext(nc) as tc:
        with ExitStack() as stack:
            def pool(name, bufs, space=None):
                kw = {"space": space} if space else {}
                return stack.enter_context(
                    tc.tile_pool(name=name, bufs=bufs, **kw))

            wpool = pool("wpool", 1)
            cpool = pool("cpool", 1)
            xpool = pool("xpool", 4)
            e0pool = pool("e0pool", 3)
            ebpool = pool("ebpool", 3)
            e1pool = pool("e1pool", 3)
            e2pool = pool("e2pool", 3)
            y0apool = pool("y0apool", 3)
            y0bpool = pool("y0bpool", 3)
            y1pool = pool("y1pool", 3)
            y2pool = pool("y2pool", 3)
            r1pool = pool("r1pool", 3)
            opool = pool("opool", 1)
            pmain = pool("pmain", 1, "PSUM")
            pspill = pool("pspill", 1, "PSUM")
            pp3 = pool("pp3", 1, "PSUM")
            # startup DMAs split across the two HWDGE queues (SP + ACT) so
            # tile 0 can start after ~max(x0, w_s0+c) instead of the sum.
            w = wpool.tile([128, WPS * N_SPECIES], BF16)
            c = cpool.tile([128, CPS * N_SPECIES], F32)
            s0 = tiles[0][0]

            def wcol(s, off, n):
                return w[:, s * WPS + off: s * WPS + off + n]

            def ccol(s, k, parts):
                return c[0:parts, s * CPS + k: s * CPS + k + 1]

            p3 = pp3.tile([1, 512], F32)

            # PE warm-up: dummy matmuls (zero weights/data, no input deps)
            # fill the startup DMA window so the HAM un-throttles to K=8/8
            # before the first real matmul; the first real L3 matmul uses
            # start=True so the garbage accumulator is overwritten.
            wu = opool.tile([128, 512], BF16, name="wu")
            nc.vector.memset(wu[:], 0.0)
            for _ in range(10):
                nc.tensor.matmul(p3[0:1, 0:512], wu[:, 0:1], wu[:, 0:512],
                                 start=True, stop=True, skip_group_check=True)

            if tiles[0][2] < 4:
                # first tile doesn't cover all 512 cols; zero explicitly
                nc.vector.memset(p3[:], 0.0)

            RELU = mybir.ActivationFunctionType.Relu

            def celu(y_ap, p_ap, e_tile, ebias, mbias):
                # e = exp(10 z + (10b+ln a)); e' = min(e-a, 0); y = (z max -b)+e'
                nc.scalar.activation(e_tile, p_ap, EXP, bias=ebias, scale=10.0)
                nc.vector.tensor_scalar(e_tile, e_tile, ALPHA, 0.0,
                                        SUB, MIN)
                nc.vector.scalar_tensor_tensor(y_ap, p_ap, mbias, e_tile,
                                               MAX, ADD)

            def celu_b(y_ap, p_ap, e_tile, r_tile, ebias, rbias):
                # structure B: r = relu(z+b) [ACT]; e' = min(e-a,0) [DVE 4x];
                # y = r + e' [DVE 4x, all-sbuf bf16] -> y = celu(t) (unshifted)
                nc.scalar.activation(e_tile, p_ap, EXP, bias=ebias, scale=10.0)
                nc.scalar.activation(r_tile, p_ap, RELU, bias=rbias, scale=1.0)
                nc.vector.tensor_scalar(e_tile, e_tile, ALPHA, 0.0,
                                        SUB, MIN)
                nc.vector.tensor_tensor(y_ap, r_tile, e_tile, ADD)

            # --- 3-stage skewed software pipeline ----------------------
            # step i: DMA x(i+1) | PE: L0(i), L1(i-1), L2(i-2), L3(i-3)
            first_mm = [True]
            xts_t = {}
            state = {}

            def dma_stage(i, eng=None):
                s, a0, na = tiles[i]
                N = na * 128
                xts = xpool.tile([128, 3 * 1024], BF16, name="xts")
                dst = xts[:, 0: 3 * N].rearrange("p (f a m) -> p f a m",
                                                 f=3, a=na, m=128)
                (eng or nc.sync).dma_start(dst, xt[:, :, a0: a0 + na, :])
                xts_t[i] = xts

            def l0_stage(i):
                s, a0, na = tiles[i]
                N = na * 128
                halves = [(k, min(na - k, 4)) for k in range(0, na, 4)]
                xts = xts_t.pop(i)
                p0 = pmain.tile([128, 1024], F32, name="p0")
                ps = pspill.tile([64, 512], F32, name="ps")
                for fc in range(3):
                    for hk, (h0, hna) in enumerate(halves):
                        rhs = xts[:, fc * N + h0 * 128:
                                  fc * N + (h0 + hna) * 128]
                        nc.tensor.matmul(
                            p0[:, h0 * 128: (h0 + hna) * 128],
                            wcol(s, fc * 128, 128), rhs,
                            start=(fc == 0), stop=(fc == 2),
                            skip_group_check=True)
                for fc in range(3):
                    for hk, (h0, hna) in enumerate(halves):
                        rhs = xts[:, fc * N + h0 * 128:
                                  fc * N + (h0 + hna) * 128]
                        nc.tensor.matmul(
                            ps[32 * hk: 32 * hk + 32, 0: hna * 128],
                            wcol(s, 384 + fc * 32, 32), rhs,
                            start=(fc == 0), stop=(fc == 2),
                            skip_group_check=True)

                e0 = e0pool.tile([128, 1024], BF16, name="e0")
                y0a = y0apool.tile([128, 1024], BF16, name="y0a")
                celu(y0a[:, 0:N], p0[:, 0:N], e0[:, 0:N],
                     ccol(s, 0, 128), ccol(s, 2, 128))

                eb = ebpool.tile([64, 512], BF16, name="eb")
                y0b = y0bpool.tile([64, 512], BF16, name="y0b")
                if na == 8:
                    celu(y0b[0:64, 0:512], ps[0:64, 0:512], eb[0:64, 0:512],
                         ccol(s, 1, 64), ccol(s, 3, 64))
                else:
                    for hk, (h0, hna) in enumerate(halves):
                        hn = hna * 128
                        celu(y0b[32 * hk: 32 * hk + 32, 0:hn],
                             ps[32 * hk: 32 * hk + 32, 0:hn],
                             eb[32 * hk: 32 * hk + 32, 0:hn],
                             ccol(s, 1, 32), ccol(s, 3, 32))
                state[i] = (y0a, y0b)

            def l1_stage(i):
                s, a0, na = tiles[i]
                N = na * 128
                halves = [(k, min(na - k, 4)) for k in range(0, na, 4)]
                y0a, y0b = state.pop(i)
                p1 = pmain.tile([128, 1024], F32, name="p1")
                for hk, (h0, hna) in enumerate(halves):
                    nc.tensor.matmul(
                        p1[:, h0 * 128: (h0 + hna) * 128],
                        wcol(s, 480, 128),
                        y0a[:, h0 * 128: (h0 + hna) * 128],
                        start=True, stop=False, skip_group_check=True)
                for hk, (h0, hna) in enumerate(halves):
                    nc.tensor.matmul(
                        p1[:, h0 * 128: (h0 + hna) * 128],
                        wcol(s, 608, 128)[32 * hk: 32 * hk + 32, :],
                        y0b[32 * hk: 32 * hk + 32, 0: hna * 128],
                        start=False, stop=True, skip_group_check=True)

                e1 = e1pool.tile([128, 1024], BF16, name="e1")
                r1 = r1pool.tile([128, 1024], BF16, name="r1")
                y1 = y1pool.tile([128, 1024], BF16, name="y1")
                celu_b(y1[:, 0:N], p1[:, 0:N], e1[:, 0:N], r1[:, 0:N],
                       ccol(s, 4, 128), ccol(s, 5, 128))
                state[i] = y1

            def l2_stage(i):
                s, a0, na = tiles[i]
                N = na * 128
                halves = [(k, min(na - k, 4)) for k in range(0, na, 4)]
                y1 = state.pop(i)
                p2 = pmain.tile([128, 1024], F32, name="p2")
                for hk, (h0, hna) in enumerate(halves):
                    nc.tensor.matmul(
                        p2[0:96, h0 * 128: (h0 + hna) * 128],
                        wcol(s, 736, 96),
                        y1[:, h0 * 128: (h0 + hna) * 128],
                        start=True, stop=True, skip_group_check=True)

                e2 = e2pool.tile([96, 1024], BF16, name="e2")
                y2 = y2pool.tile([96, 1024], BF16, name="y2")
                celu(y2[:, 0:N], p2[0:96, 0:N], e2[:, 0:N],
                     ccol(s, 6, 96), ccol(s, 7, 96))
                state[(2, i)] = y2

            def l3_stage(i):
                s, a0, na = tiles[i]
                halves = [(k, min(na - k, 4)) for k in range(0, na, 4)]
                y2 = state.pop((2, i))
                for hk, (h0, hna) in enumerate(halves):
                    nc.tensor.matmul(
                        p3[0:1, 0: hna * 128],
                        wcol(s, 832, 1)[0:96, :],
                        y2[:, h0 * 128: (h0 + hna) * 128],
                        start=first_mm[0],
                        stop=(i == ntiles - 1 and hk == len(halves) - 1),
                        skip_group_check=True)
                    first_mm[0] = False

            # startup: x0 on the ACT hwdge queue, weights/consts on SP —
            # the two transfers run in parallel so tile 0 starts earliest.
            dma_stage(0, eng=nc.scalar)
            nc.sync.dma_start(c[:], cp[:])
            nc.sync.dma_start(w[:, s0 * WPS: (s0 + 1) * WPS],
                              wp[:, s0 * WPS: (s0 + 1) * WPS])
            for sq in range(N_SPECIES):
                if sq != s0:
                    nc.scalar.dma_start(w[:, sq * WPS: (sq + 1) * WPS],
                                        wp[:, sq * WPS: (sq + 1) * WPS])
            for step in range(ntiles + 3):
                if step + 1 < ntiles:
                    dma_stage(step + 1)
                if step < ntiles:
                    l0_stage(step)
                if 0 <= step - 1 < ntiles:
                    l1_stage(step - 1)
                if 0 <= step - 2 < ntiles:
                    l2_stage(step - 2)
                if 0 <= step - 3 < ntiles:
                    l3_stage(step - 3)

            t3 = opool.tile([1, 512], F32)
            nc.scalar.copy(t3[:], p3[:])
            f01 = opool.tile([1, 128], F32)
            nc.vector.tensor_add(f01[:], t3[0:1, 0:128], t3[0:1, 128:256])
            f23 = opool.tile([1, 128], F32)
            nc.vector.tensor_add(f23[:], t3[0:1, 256:384], t3[0:1, 384:512])
            fo = opool.tile([1, 128], F32)
            nc.vector.tensor_add(fo[:], f01[:], f23[:])
            nc.sync.dma_start(yo[:], fo[:])

    nc.compile()
    return nc


def _celu64(z):
    return np.where(z > 0, z, ALPHA * np.expm1(z / ALPHA))


def kernel(fullaev, species, W0, b0, W1, b1, W2, b2, W3, b3):
    import ml_dtypes
    from concourse import bass_utils

    BF = ml_dtypes.bfloat16

    fullaev = np.ascontiguousarray(np.asarray(fullaev, dtype=np.float32))
    species = np.asarray(species, dtype=np.int32)
    Ws = [np.asarray(w, dtype=np.float32) for w in (W0, W1, W2, W3)]
    bs = [np.asarray(b, dtype=np.float32) for b in (b0, b1, b2, b3)]

    # --- species grouping: per-core slot assignment ---------------------
    ids = [np.where(species == s)[0] for s in range(N_SPECIES)]
    n = [len(i) for i in ids]
    G = []
    for s in range(N_SPECIES):
        g = -(-n[s] // N_CORES) if n[s] else 0
        g += g % 2
        G.append(g)
    S = sum(G)
    key = tuple(G)
    if key not in _progs:
        _progs[key] = _build_program(G, S)
    nc = _progs[key]

    # --- fold constants (float64) ---------------------------------------
    corr = np.zeros(N_SPECIES)
    Kdum = np.zeros(N_SPECIES)
    cpack = np.zeros((128, CPS * N_SPECIES), np.float32)
    wpack = np.zeros((128, WPS * N_SPECIES), np.float32)
    for s in range(N_SPECIES):
        w0, w1, w2, w3 = (w[s].astype(np.float64) for w in Ws)
        bb0, bb1, bb2, bb3 = (b[s].astype(np.float64) for b in bs)
        c1 = bb1 + w1 @ bb0
        c2 = bb2  # L1 site uses structure B (stores unshifted celu)
        corr[s] = bb3[0] + w3[0] @ c2
        y = _celu64(bb0)
        y = _celu64(w1 @ y + bb1)
        y = _celu64(w2 @ y + bb2)
        Kdum[s] = w3[0] @ y + bb3[0]

        cb = s * CPS
        # spill exp/max biases: b0[128:160] tiled twice over 64 partitions
        b0sp = np.concatenate([bb0[128:160], bb0[128:160]])
        cpack[:, cb + 0] = 10.0 * bb0[:128] + LNA
        cpack[:64, cb + 1] = 10.0 * b0sp + LNA
        cpack[:, cb + 2] = -bb0[:128]
        cpack[:64, cb + 3] = -b0sp
        cpack[:, cb + 4] = 10.0 * c1 + LNA
        cpack[:, cb + 5] = c1  # structure B relu bias (+c1)
        cpack[:96, cb + 6] = 10.0 * c2 + LNA
        cpack[:96, cb + 7] = -c2

        wb = s * WPS
        # L0 main: 3 chunks of [128in, 128out]
        for fc in range(3):
            wpack[:, wb + fc * 128: wb + (fc + 1) * 128] = (
                Ws[0][s][:128, fc * 128:(fc + 1) * 128].T)
        # L0 spill: 3 chunks of [128in, 32out]
        for fc in range(3):
            wpack[:, wb + 384 + fc * 32: wb + 384 + (fc + 1) * 32] = (
                Ws[0][s][128:160, fc * 128:(fc + 1) * 128].T)
        wpack[:, wb + 480: wb + 608] = Ws[1][s][:, :128].T
        wpack[:32, wb + 608: wb + 736] = Ws[1][s][:, 128:].T
        wpack[32:64, wb + 608: wb + 736] = Ws[1][s][:, 128:].T
        wpack[:, wb + 736: wb + 832] = Ws[2][s].T
        wpack[:96, wb + 832] = Ws[3][s][0, :]

    wpack = wpack.astype(BF)
    # --- per-core transposed, species-sorted AEV blocks (bf16) ----------
    in_maps = []
    dummy_counts = np.zeros((N_CORES, N_SPECIES), np.int64)
    for cid in range(N_CORES):
        xtc = np.zeros((128, 3, S, 128), BF)
        slot0 = 0
        for s in range(N_SPECIES):
            mine = ids[s][cid::N_CORES]
            nr = len(mine)
            dummy_counts[cid, s] = G[s] - nr
            if nr:
                g = fullaev[:, mine, :]               # [128, nr, 384]
                t = g.transpose(2, 1, 0)              # [384, nr, 128]
                xtc[:, :, slot0: slot0 + nr, :] = (
                    t.reshape(3, 128, nr, 128).transpose(1, 0, 2, 3)
                ).astype(BF)
            slot0 += G[s]
        in_maps.append({"xt": xtc, "wp": wpack, "cp": cpack})

    if TRACE:
        _maybe_register_ntff_hook()
    res = bass_utils.run_bass_kernel_spmd(
        nc, in_maps, core_ids=list(range(N_CORES)), trace=TRACE
    )
    LAST["exec_time_ns"] = res.exec_time_ns
    LAST["trace"] = res.instructions_and_trace[1] if res.instructions_and_trace else None

    total_corr = 0.0
    for s in range(N_SPECIES):
        total_corr += N_CORES * G[s] * corr[s] - dummy_counts[:, s].sum() * Kdum[s]
    out = np.zeros(128, np.float64)
    for cid in range(N_CORES):
        out += res.results[cid]["yo"][0].astype(np.float64)
    out += total_corr
    return out.astype(np.float32)

